# revision 1
# baseline (speedup 1.0000x reference)
"""Trainium2 Bass kernel for nn_DNN_Model_33852932227151.

Per-sample pipeline (see reference):
  theta1 = MLP(sample1)            303 -> 1024 -> 1024 -> 512 -> 264
  F1, F2 normalized precoders      (cols 200:264)
  theta  = unit-modulus phases     (cols 0:200 as complex [100])
  CCC_bc = Re(theta^H T_bc theta) / 1e-15 ; scale = rsqrt(max(max_c CCC, 1))
  out    = [Re(theta*scale), Im(theta*scale), Re F1, Im F1, Re F2, Im F2]

Sharding: pure data parallel over batch: 2048 = 8 cores x 256 samples.

Design notes (v4):
  - All math on the theta->quad path is fp32: CCC = quad*1e15 clamped at 1
    means a bf16 contraction (abs err ~0.3) flips the clamp for samples
    whose true max-quad lands near 0 -> catastrophically wrong scale.
  - Per-core HBM floor ~91 MB (82 MB T + 8.6 MB weights) ~ 255us. All input
    DMAs ride the sync HWDGE ring in just-in-time order; T pair-chunks queue
    FIFO behind the weights; outputs ride the scalar ring.
  - T_real/T_imag slices load into ONE pair tile [128, 2, 25, 100] so a
    single DVE multiply + single ACT accumulate handle Re+Im per chunk
    (Re(quad) = sum Tr.o1 + Ti.o2 falls out of one accumulator). 4 chunk
    buffers cover the 3-stage DMA->mul->accum pipeline at DMA rate.
  - Outers live in a pair tile o12 [128, 2, 50, 100] per (group, h) unit;
    build = [tmp=ab^T, o2=ba^T, o1=ss^T (s=a+b)] on GpSimd (its ~6us fixed
    Q7 cost wants big ops) + [o1-=o2(pre), o12-=tmp broadcast] on DVE.
    Unit 0's build runs DVE-heavy since nothing overlaps it.
  - MLP: L1/L2 full batch (N=256; fp32 matmul cost is per-instruction
    overhead dominated, so N=128 splitting loses), L3/L4 split per group
    to reach theta(g0) sooner; theta/F normalization emitted in phase 2 so
    the DVE starts streaming without queuing behind group 1's theta.
"""

import os
import threading

import numpy as np

import concourse.bass as bass
from concourse import bacc
import concourse.mybir as mybir
import concourse.tile as tile
from concourse.bass_utils import run_bass_kernel_spmd

F32 = mybir.dt.float32

# ---- problem constants (hardcoded per harness contract) ----
B = 2048
N_CORES = 8
B_LOC = B // N_CORES          # 256 samples per core
DIN = 303
H1, H2, H3 = 1024, 1024, 512
DOUT = 264
NRIS = 100
C = 4
MN = 16
INV_THRESH = 1.0e15           # 1 / THRESH_W

N_GROUPS = B_LOC // 128       # 2 sample groups of 128 per core

OROWS = 50                    # outer-tile rows per unit
N_H = NRIS // OROWS           # units per group
CROWS = 25                    # chunk rows per DMA/mul/accum
N_CH = OROWS // CROWS         # chunk h-steps per unit
TCH_BUFS = int(os.environ.get("KERNEL_TCH_BUFS", "4"))
# engines for build ops [ab, ba, ss, o1-=o2, o12-=ab]: the big pair-sub
# alternates GP/DVE per unit to balance both engines' load
BUILD_ENGS_EVEN = os.environ.get("KERNEL_BUILD_EVEN", "gp,gp,gp,dve,dve").split(",")
BUILD_ENGS_ODD = os.environ.get("KERNEL_BUILD_ODD", "gp,gp,gp,dve,gp").split(",")
# and for unit 0 (critical path, everything else idle)
BUILD_ENGS0 = os.environ.get("KERNEL_BUILD0", "gp,dve,dve,dve,dve").split(",")
# debug bisect: "full" | "noquad" (skip T contraction; scale=1)
STAGE = os.environ.get("KERNEL_STAGE", "full")


def build_nc():
    nc = bacc.Bacc(trn_type="TRN2", debug=False)

    s1 = nc.declare_dram_parameter("sample1", [B_LOC, DIN], F32, isOutput=False)
    t_re = nc.declare_dram_parameter("T_real", [B_LOC, C, NRIS, NRIS], F32, isOutput=False)
    t_im = nc.declare_dram_parameter("T_imag", [B_LOC, C, NRIS, NRIS], F32, isOutput=False)
    w1 = nc.declare_dram_parameter("W1", [DIN, H1], F32, isOutput=False)
    b1 = nc.declare_dram_parameter("b1", [H1], F32, isOutput=False)
    w2 = nc.declare_dram_parameter("W2", [H1, H2], F32, isOutput=False)
    b2 = nc.declare_dram_parameter("b2", [H2], F32, isOutput=False)
    w3 = nc.declare_dram_parameter("W3", [H2, H3], F32, isOutput=False)
    b3 = nc.declare_dram_parameter("b3", [H3], F32, isOutput=False)
    w4 = nc.declare_dram_parameter("W4", [H3, DOUT], F32, isOutput=False)
    b4 = nc.declare_dram_parameter("b4", [DOUT], F32, isOutput=False)
    out = nc.declare_dram_parameter("out", [B_LOC, DOUT], F32, isOutput=True)

    ident_dram = nc.inline_tensor(np.eye(128, dtype=np.float32), name="ident128")

    with tile.TileContext(nc) as tc:
        _emit(tc, s1, t_re, t_im, (w1, b1), (w2, b2), (w3, b3), (w4, b4),
              out, ident_dram)
    nc.compile()
    return nc


def _emit(tc, s1, t_re, t_im, l1, l2, l3, l4, out, ident_dram):
    nc = tc.nc
    w1, b1 = l1
    w2, b2 = l2
    w3, b3 = l3
    w4, b4 = l4
    TT = mybir.AluOpType

    with (
        tc.tile_pool(name="consts", bufs=1) as consts,
        tc.tile_pool(name="acts", bufs=1) as acts,
        tc.tile_pool(name="theta", bufs=1) as theta_pool,
        tc.tile_pool(name="tsc", bufs=2) as tsc_pool,
        tc.tile_pool(name="tch", bufs=TCH_BUFS) as tch_pool,
        tc.tile_pool(name="psmm", bufs=6, space="PSUM") as psmm,
        tc.tile_pool(name="pstr", bufs=2, space="PSUM") as pstr,
    ):
        ident = consts.tile([128, 128], F32)
        thp = acts.tile([128, 3, B_LOC], F32)  # [:,0]=re, [:,1]=im, [:,2]=F(64)
        a_fm = theta_pool.tile([128, B_LOC], F32)
        b_fm = theta_pool.tile([128, B_LOC], F32)
        nc.vector.memset(thp[64:128, 2, :], 0.0)
        nc.vector.memset(a_fm, 0.0)
        nc.vector.memset(b_fm, 0.0)

        gstate = [dict() for _ in range(N_GROUPS)]

        # ============ phase 1: MLP through L4 (weights pool scope) ============
        with tc.tile_pool(name="wpool", bufs=1) as wpool:
            # Allocate wpool tiles in order of LAST READ (earliest-freed
            # first): phase 2's ob pools reuse this arena, and Tile WARs each
            # ob tile only against the wpool tiles at the same addresses --
            # so tmp/o12-A land on L1/L2-era tiles and unblock right after
            # L2/L3 instead of waiting for the whole MLP.
            x0 = wpool.tile([128, 3, B_LOC], F32)      # free after L1
            w1s = wpool.tile([128, 3, H1], F32)
            b1s = wpool.tile([128, 8], F32)
            w2s = wpool.tile([128, 8, H2], F32)        # free after L2
            h1t = wpool.tile([128, 8, B_LOC], F32)
            b2s = wpool.tile([128, 8], F32)
            w3s = wpool.tile([128, 8, H3], F32)        # free after L3
            h2t = wpool.tile([128, 8, B_LOC], F32)
            b3s = wpool.tile([128, 4], F32)
            w4s = wpool.tile([128, 4, DOUT], F32)      # free after L4
            h3t = wpool.tile([128, 4, B_LOC], F32)
            b4s = wpool.tile([128, 3], F32)

            nc.sync.dma_start(out=ident, in_=ident_dram[:, :])
            s1_nats = []
            for bt in range(2):
                s1_nat = tsc_pool.tile([128, DIN], F32, tag="s1nat")
                nc.sync.dma_start(out=s1_nat, in_=s1[bt * 128:(bt + 1) * 128, :])
                s1_nats.append(s1_nat)

            nc.vector.memset(w1s[:, 2, :], 0.0)
            nc.sync.dma_start(out=w1s[:, 0, :], in_=w1[0:128, :])
            nc.sync.dma_start(out=w1s[:, 1, :], in_=w1[128:256, :])
            nc.sync.dma_start(out=w1s[0:47, 2, :], in_=w1[256:303, :])
            nc.sync.dma_start(out=b1s, in_=b1[:].rearrange("(o p) -> p o", p=128))
            nc.sync.dma_start(out=b2s, in_=b2[:].rearrange("(o p) -> p o", p=128))
            nc.sync.dma_start(out=b3s, in_=b3[:].rearrange("(o p) -> p o", p=128))
            nc.sync.dma_start(out=b4s[0:100, 0:1], in_=b4[0:100, None])
            nc.sync.dma_start(out=b4s[0:100, 1:2], in_=b4[100:200, None])
            nc.sync.dma_start(out=b4s[0:64, 2:3], in_=b4[200:264, None])
            # per-k-block weight loads: contiguous rows -> cheap descriptors,
            # and each k-block lands as soon as possible (JIT arrival)
            for k in range(8):
                nc.sync.dma_start(out=w2s[:, k, :], in_=w2[k * 128:(k + 1) * 128, :])
            for k in range(8):
                nc.sync.dma_start(out=w3s[:, k, :], in_=w3[k * 128:(k + 1) * 128, :])
            for k in range(4):
                nc.sync.dma_start(out=w4s[:, k, :], in_=w4[k * 128:(k + 1) * 128, :])

            nc.vector.memset(x0[:, 2, :], 0.0)

            def dense(gs, n_cols, in_tile, n_k, ws, n_m, bias_s, relu, out_tile,
                      m_widths=None):
                for mo in range(n_m):
                    if m_widths is None:
                        mw, m_lo = 128, mo * 128
                    else:
                        m_lo, mw = m_widths[mo]
                    ps = psmm.tile([128, B_LOC], F32, tag="mm")
                    for k in range(n_k):
                        nc.tensor.matmul(
                            ps[0:mw, 0:n_cols],
                            lhsT=ws[:, k, m_lo:m_lo + mw],
                            rhs=in_tile[:, k, gs],
                            start=(k == 0),
                            stop=(k == n_k - 1),
                        )
                    if relu:
                        nc.scalar.activation(
                            out=out_tile[0:mw, mo, gs], in_=ps[0:mw, 0:n_cols],
                            func=mybir.ActivationFunctionType.Relu,
                            bias=bias_s[0:mw, mo:mo + 1], scale=1.0)
                    else:
                        nc.vector.tensor_scalar(
                            out=out_tile[0:mw, mo, gs], in0=ps[0:mw, 0:n_cols],
                            scalar1=bias_s[0:mw, mo:mo + 1], scalar2=None,
                            op0=mybir.AluOpType.add)

            for bt in range(2):
                for ft in range(3):
                    w = min(128, DIN - ft * 128)
                    ps = pstr.tile([128, 128], F32, tag="tr")
                    nc.tensor.transpose(ps[0:w, :],
                                        s1_nats[bt][:, ft * 128:ft * 128 + w],
                                        ident)
                    nc.scalar.copy(out=x0[0:w, ft, bt * 128:(bt + 1) * 128],
                                   in_=ps[0:w, :])
            full = slice(0, B_LOC)
            dense(full, B_LOC, x0, 3, w1s, 8, b1s, True, h1t)
            dense(full, B_LOC, h1t, 8, w2s, 8, b2s, True, h2t)
            dense(full, B_LOC, h2t, 8, w3s, 4, b3s, True, h3t)
            # L4 split per group so theta(g0) is ready sooner
            for g in range(N_GROUPS):
                gs = slice(g * 128, (g + 1) * 128)
                dense(gs, 128, h3t, 4, w4s, 3, b4s, False, thp,
                      m_widths=[(0, 100), (100, 100), (200, 64)])

        # ============ phase 2: theta + streaming contraction ============
        with (
            tc.tile_pool(name="ob1", bufs=1) as ob1,   # tmp (= ab^T), lowest addr
            tc.tile_pool(name="ob2", bufs=2) as ob2,   # o12 pair tiles
        ):
            def emit_theta_group(g):
                """unit-modulus theta + transposes + F norm (no wpool refs)"""
                gs = slice(g * 128, (g + 1) * 128)
                st = gstate[g]
                p_re = thp[0:100, 0, gs]
                p_im = thp[0:100, 1, gs]
                sq = tsc_pool.tile([100, 128], F32, tag="sq")
                sq2 = tsc_pool.tile([100, 128], F32, tag="sq2")
                nc.vector.tensor_mul(sq, p_re, p_re)
                nc.vector.tensor_mul(sq2, p_im, p_im)
                nc.vector.tensor_add(sq, sq, sq2)
                nc.scalar.sqrt(sq, sq)
                nc.vector.reciprocal(sq, sq)               # 1/|theta|
                nc.vector.tensor_mul(a_fm[0:100, gs], p_re, sq)
                nc.vector.tensor_mul(b_fm[0:100, gs], p_im, sq)

                def to_sample_major(src_fm, np_, tag):
                    ps = pstr.tile([128, 128], F32, tag="tr")
                    nc.tensor.transpose(ps, src_fm, ident)
                    dst = theta_pool.tile([128, np_], F32, tag=tag)
                    nc.scalar.copy(out=dst, in_=ps[:, 0:np_])
                    return dst

                a_pack = to_sample_major(a_fm[:, gs], 100, f"apack{g}")
                b_pack = to_sample_major(b_fm[:, gs], 100, f"bpack{g}")
                f_pack = to_sample_major(thp[:, 2, gs], 64, f"fpack{g}")
                st["a"], st["b"] = a_pack, b_pack
                s_pack = theta_pool.tile([128, NRIS], F32, tag=f"spack{g}")
                nc.vector.tensor_add(s_pack, a_pack, b_pack)
                st["s"] = s_pack

                fsq = tsc_pool.tile([128, 2, 32], F32, tag="fsq")
                f_v = f_pack[:].rearrange("p (g2 i) -> p g2 i", g2=2)
                nc.vector.tensor_mul(fsq, f_v, f_v)
                fnorm = tsc_pool.tile([128, 2], F32, tag="fnorm")
                nc.vector.reduce_sum(fnorm, fsq, axis=mybir.AxisListType.X)
                nc.scalar.activation(out=fnorm, in_=fnorm,
                                     func=mybir.ActivationFunctionType.Sqrt,
                                     scale=0.5)
                nc.vector.reciprocal(fnorm, fnorm)
                fhat = theta_pool.tile([128, 2, 32], F32, tag=f"fhat{g}")
                nc.vector.tensor_mul(fhat, f_v,
                                     fnorm[:, :, None].to_broadcast((128, 2, 32)))
                nc.scalar.dma_start(out=out[gs, 200:264],
                                    in_=fhat[:].rearrange("p g2 i -> p (g2 i)"))
                parts = theta_pool.tile([128, C, N_H * N_CH], F32, tag=f"parts{g}")
                st["parts"] = parts

            def eng(name):
                return nc.gpsimd if name == "gp" else nc.vector

            def build_unit_ops(g, h, engs):
                """5 closures building o1 = aa^T+bb^T, o2 = ba^T-ab^T."""
                st = gstate[g]
                a, b, s = st["a"], st["b"], st["s"]
                hs = slice(h * OROWS, (h + 1) * OROWS)
                sh3 = (128, OROWS, NRIS)
                sh4 = (128, 2, OROWS, NRIS)
                o12 = ob2.tile([128, 2, OROWS, NRIS], F32, tag="o12")
                o1, o2 = o12[:, 0], o12[:, 1]
                tmp = ob1.tile([128, OROWS, NRIS], F32, tag="tmp")

                def mul_op(e, dst, col, row):
                    def f():
                        eng(e).tensor_mul(dst, col[:, hs, None].to_broadcast(sh3),
                                          row[:, None, :].to_broadcast(sh3))
                    return f

                def sub_op(e, dst, x, y):
                    def f():
                        eng(e).tensor_tensor(dst, x, y, TT.subtract)
                    return f

                ops = [
                    mul_op(engs[0], tmp, a, b),   # tmp = a_n b_m
                    mul_op(engs[1], o2, b, a),    # o2 = b_n a_m
                    mul_op(engs[2], o1, s, s),    # o1 = s_n s_m
                    sub_op(engs[3], o1, o1, o2),  # o1 = ss - ba  (pre-sub o2)
                    sub_op(engs[4], o12, o12,
                           tmp[:, None, :, :].to_broadcast(sh4)),  # both -= ab
                ]
                return ops, o12

            def emit_finale(g):
                gs = slice(g * 128, (g + 1) * 128)
                st = gstate[g]
                th = theta_pool.tile([128, 2, NRIS], F32, tag=f"th{g}")
                mx = tsc_pool.tile([128, 1], F32, tag="mx")
                if STAGE == "noquad":
                    nc.vector.memset(mx, 1.0)
                else:
                    ccc = tsc_pool.tile([128, C], F32, tag="ccc")
                    nc.vector.reduce_sum(ccc, st["parts"], axis=mybir.AxisListType.X)
                    nc.vector.reduce_max(mx, ccc, axis=mybir.AxisListType.X)
                    # scale = rsqrt(max(mx*1e15, 1)) = rsqrt(1e15*max(mx, 1e-15))
                    nc.vector.tensor_scalar(out=mx, in0=mx,
                                            scalar1=1.0 / INV_THRESH,
                                            scalar2=None, op0=mybir.AluOpType.max)
                    nc.scalar.activation(out=mx, in_=mx,
                                         func=mybir.ActivationFunctionType.Sqrt,
                                         scale=INV_THRESH)
                    nc.vector.reciprocal(mx, mx)
                nc.vector.tensor_scalar_mul(th[:, 0, :], st["a"], mx)
                nc.vector.tensor_scalar_mul(th[:, 1, :], st["b"], mx)
                nc.scalar.dma_start(out=out[gs, 0:200],
                                    in_=th[:].rearrange("p r n -> p (r n)"))

            emit_theta_group(0)
            units = [(g, h) for g in range(N_GROUPS) for h in range(N_H)]
            ops0, o12 = build_unit_ops(*units[0], BUILD_ENGS0)
            for f in ops0:
                f()

            for ui, (g, h) in enumerate(units):
                if ui == 1:
                    emit_theta_group(1)
                pending = []
                if ui + 1 < len(units):
                    engs = BUILD_ENGS_ODD if (ui + 1) % 2 else BUILD_ENGS_EVEN
                    pending, next_o12 = build_unit_ops(*units[ui + 1], engs)
                    # GP muls go to the engine queue up front (GP is idle and
                    # slow; its 3 muls must finish within this unit's window);
                    # DVE subs stay deferred behind most of this unit's muls.
                    while pending and engs[5 - len(pending)] == "gp":
                        pending.pop(0)()
                st = gstate[g]
                parts = st["parts"]
                slot = 0
                n_slots = C * N_CH
                for c in range(C):
                    for hc in range(N_CH):
                        r0 = h * OROWS + hc * CROWS
                        pc = tch_pool.tile([128, 2, CROWS, NRIS], F32, tag="tchunk")
                        nc.sync.dma_start(
                            out=pc[:, 0],
                            in_=t_re[g * 128:(g + 1) * 128, c, r0:r0 + CROWS, :])
                        nc.sync.dma_start(
                            out=pc[:, 1],
                            in_=t_im[g * 128:(g + 1) * 128, c, r0:r0 + CROWS, :])
                        if STAGE == "noquad":
                            continue
                        o_sl = o12[:, :, hc * CROWS:(hc + 1) * CROWS, :]
                        nc.vector.tensor_mul(pc, pc, o_sl)
                        acc = parts[:, c, h * N_CH + hc:h * N_CH + hc + 1]
                        nc.scalar.activation(
                            out=pc, in_=pc,
                            func=mybir.ActivationFunctionType.Copy,
                            bias=0.0, scale=1.0, accum_out=acc)
                        slot += 1
                        if pending and slot >= n_slots - 2:
                            pending.pop(0)()
                for f in pending:
                    f()
                if ui + 1 < len(units):
                    o12 = next_o12
                if h == N_H - 1:
                    emit_finale(g)


_NC_LOCK = threading.Lock()
_NC = None


def _get_nc():
    global _NC
    with _NC_LOCK:
        if _NC is None:
            _NC = build_nc()
    return _NC


def _shard_inputs(inputs):
    in_maps = []
    for i in range(N_CORES):
        bs = slice(i * B_LOC, (i + 1) * B_LOC)
        in_maps.append({
            "sample1": np.ascontiguousarray(inputs["sample1"][bs]),
            "T_real": np.ascontiguousarray(inputs["T_real"][bs]),
            "T_imag": np.ascontiguousarray(inputs["T_imag"][bs]),
            "W1": np.asarray(inputs["W1"]), "b1": np.asarray(inputs["b1"]),
            "W2": np.asarray(inputs["W2"]), "b2": np.asarray(inputs["b2"]),
            "W3": np.asarray(inputs["W3"]), "b3": np.asarray(inputs["b3"]),
            "W4": np.asarray(inputs["W4"]), "b4": np.asarray(inputs["b4"]),
        })
    return in_maps


def run_on_hw(inputs, trace=False, **kwargs):
    nc = _get_nc()
    res = run_bass_kernel_spmd(nc, _shard_inputs(inputs),
                               list(range(N_CORES)), trace=trace, **kwargs)
    full = np.concatenate([res.results[i]["out"] for i in range(N_CORES)], axis=0)
    return full, res


def kernel(**inputs) -> np.ndarray:
    full, _ = run_on_hw(inputs, trace=False)
    return full.astype(np.float32)

